# revision 78
# baseline (speedup 1.0000x reference)
"""Trainium2 Bass kernel for nn_EntropyLM (wavelet-coeff mixer + chunked MHA + output proj).

Strategy: data-parallel over the 16 independent (batch x chunk) blocks, 2 per
NeuronCore.  The numerically-critical path (wavelet coeffs, mixer, residual
stream, output projection) runs in fp16 on the PE (same speed as bf16, 8x the
mantissa); the error-tolerant bulk (q/k/v projections, attention scores, PV,
attention-out projection) runs in fp8 e4m3 with DoubleRow perf mode, which
contracts K=256 per instruction at 0.5 cycles/row -- 4x bf16 matmul
throughput in the HW cost model.

Per-tensor power-of-two scales keep fp8 operands in [~1, 200]; all scale
corrections are folded into PSUM-evacuation ops that are needed anyway.

Layouts per chunk (CHUNK=1024 tokens, H=1024 features):
  * "T" tensors are feature-major [feat_part, ktile, token]; "N" tensors are
    token-major [token_part, ttile, feat].
  * Attention-out (ocat, token-major fp8) is transposed for the wo matmul by
    viewing fp8 pairs as uint16 through the DMA xbar transpose; the row
    permutation this induces on the contraction index is compensated by
    pre-permuting wo's rows on the host (wo8p).
  * The softmax denominator comes from a 1-column DoubleRow matmul against a
    constant 0.125 vector (reusing the PV lhsT weights); normalization is a
    per-partition scale on the PV evacuation.

The two chunks per core are software-pipelined by emission order: chunk B's
PE-heavy projection tiles are drained as filler between chunk A's Act-bound
attention pieces so the PE never idles waiting on exp().
"""

import numpy as np
import ml_dtypes

B, S, H, G, W = 4, 4096, 1024, 256, 8
CHUNK = 1024
NUM_HEADS = 4
HD = H // NUM_HEADS          # 256 per-head dim
HM = H // 2                  # 512 mixer hidden
N_CHUNKS = B * (S // CHUNK)  # 16 independent chunks
N_CORES = 8
CPC = N_CHUNKS // N_CORES    # 2 chunks per core
NT = CHUNK // 128            # 8 token tiles
KH = H // 128                # 8 feature tiles (H)
KM = HM // 128               # 4 feature tiles (HM)
EPS = 1e-5
BF16 = ml_dtypes.bfloat16
F8 = ml_dtypes.float8_e4m3
F16 = np.float16

# fp8 scales (powers of two; folded into evacuation ops)
S_W8 = 1024.0    # wq/wk/wv/wo weight scale
S_M8 = 64.0      # mix8 activation scale
S_Q8 = 128.0     # q/k fp8 scale
S_V8 = 128.0     # v fp8 scale
S_ET = 16.0      # exp(score) scale
C_ONE = 0.5      # denominator ones value -> ocat = (S_V8/C_ONE) * o = 256*o
S_O8 = S_V8 / C_ONE              # 1024
INV_WO = 1.0 / (S_O8 * S_W8)     # 2^-20

_COMPILED = None
STAGE_MARKS = []


def _build(debug=False):
    import concourse.bass as bass  # noqa: F401
    import concourse.tile as tile
    from concourse import bacc, mybir

    f8 = mybir.dt.float8e4
    u16 = mybir.dt.uint16
    fp16 = mybir.dt.float16
    f32 = mybir.dt.float32
    Alu = mybir.AluOpType
    Act = mybir.ActivationFunctionType
    DR = mybir.MatmulPerfMode.DoubleRow

    nc = bacc.Bacc("TRN2", target_bir_lowering=False, debug=False,
                   enable_asserts=True, num_devices=N_CORES)

    # ---- DRAM tensors (per-core views; same NEFF on all 8 cores) ----
    xt = nc.dram_tensor("xt", [CPC, H, CHUNK], fp16, kind="ExternalInput")
    kernT = nc.dram_tensor("kernt", [H, W], fp16, kind="ExternalInput")
    w1a = nc.dram_tensor("w1a", [W + 1, HM], fp16, kind="ExternalInput")
    smallw = nc.dram_tensor("smallw", [128, 2 * KM + KH + G], f32,
                            kind="ExternalInput")
    w2 = nc.dram_tensor("w2", [HM, H], fp16, kind="ExternalInput")
    wq8 = nc.dram_tensor("wq8", [H, H], f8, kind="ExternalInput")
    wk8 = nc.dram_tensor("wk8", [H, H], f8, kind="ExternalInput")
    wv8 = nc.dram_tensor("wv8", [H, H], f8, kind="ExternalInput")
    wo8 = nc.dram_tensor("wo8", [H, H], f8, kind="ExternalInput")
    gw = nc.dram_tensor("gw", [H, G], fp16, kind="ExternalInput")
    y = nc.dram_tensor("y", [CPC, CHUNK, G], f32, kind="ExternalOutput")
    dbg = {}
    if debug:
        for nm, shp, dt in [
            ("dcoef", [W + 1, CHUNK], fp16),
            ("dhidT", [128, KM, CHUNK], fp16),
            ("dmix8", [128, KH, CHUNK], f8),
            ("dmixN", [128, NT, H], fp16),
            ("dqT", [128, KH, CHUNK], f8),
            ("dkT", [128, KH, CHUNK], f8),
            ("dvN", [128, NT, H], f8),
            ("det", [128, KH, CHUNK], f8),
            ("ddn", [128, NUM_HEADS, NT], f32),
            ("dotc", [128, KH, CHUNK], f8),
            ("dres", [128, NT, H], fp16),
        ]:
            dbg[nm] = nc.dram_tensor(nm, shp, dt, kind="ExternalOutput")

    with tile.TileContext(nc) as tc:
        with (
            tc.tile_pool(name="wp", bufs=1) as wp,
            tc.tile_pool(name="ws", bufs=1) as ws,
            tc.tile_pool(name="sm", bufs=2) as sm,
            tc.tile_pool(name="ps", bufs=1, space="PSUM") as ps,
        ):
            # ---------- persistent weights ----------
            kt_sb = wp.tile([128, KH, W], fp16, tag="ktw")
            nc.sync.dma_start(kt_sb[:], kernT.ap().rearrange("(i p) w -> p i w", p=128))
            w1a_sb = wp.tile([W + 1, HM], fp16, tag="w1a")
            nc.sync.dma_start(w1a_sb[:], w1a.ap())
            smallw_sb = wp.tile([128, 2 * KM + KH + G], f32, tag="smallw")
            gln_sb = smallw_sb[:, 0:KM]
            bln_sb = smallw_sb[:, KM:2 * KM]
            b2_sb = smallw_sb[:, 2 * KM:2 * KM + KH]
            bw_sb = smallw_sb[:, 2 * KM + KH:]
            w2_sb = wp.tile([128, KM, H], fp16, tag="w2s")
            wq_sb = wp.tile([128, KH, H], f8, tag="wq")
            wk_sb = wp.tile([128, KH, H], f8, tag="wk")
            wv_sb = wp.tile([128, KH, H], f8, tag="wv")
            wo_sb = wp.tile([128, KH, H], f8, tag="wo")
            gw_sb = wp.tile([128, KH, G], fp16, tag="gw")

            def load_big_weights():
                # emitted after the x-stream DMAs so they don't delay S1
                nc.scalar.dma_start(smallw_sb[:], smallw.ap())
                nc.sync.dma_start(w2_sb[:],
                                  w2.ap().rearrange("(i p) m -> p i m", p=128))
                nc.scalar.dma_start(wq_sb[:],
                                    wq8.ap().rearrange("(i p) m -> p i m", p=128))
                nc.sync.dma_start(wk_sb[:],
                                  wk8.ap().rearrange("(i p) m -> p i m", p=128))
                nc.scalar.dma_start(wv_sb[:],
                                    wv8.ap().rearrange("(i p) m -> p i m", p=128))
                nc.sync.dma_start(gw_sb[:],
                                  gw.ap().rearrange("(i p) g -> p i g", p=128))
                nc.scalar.dma_start(wo_sb[:],
                                    wo8.ap().rearrange("(i p) m -> p i m", p=128))
            ones8 = wp.tile([128, 2, 1], f8, tag="ones")
            nc.vector.memset(ones8[:], C_ONE)
            eps_sb = wp.tile([128, 1], f32, tag="eps")
            nc.vector.memset(eps_sb[:], EPS)
            lns_sb = wp.tile([128, 1], f32, tag="lns")
            nc.vector.memset(lns_sb[:], float(np.log(S_ET)))

            # ---------- per-chunk state ----------
            st = [dict() for _ in range(CPC)]

            def psum_big(n=1024):
                return ps.tile([128, n], f32, tag="big", bufs=2, name="pbig")

            def psum_st():
                return ps.tile([128, 1024], f32, tag="st", bufs=2, name="pst")

            # ----- S0+S1: stream x (both queues), wavelet coeffs -----
            def s01_load(c):
                xf = ws.tile([128, KH, CHUNK], fp16, tag=f"xet{c}", name="xf")
                for j in range(4):
                    eng = nc.sync if j % 2 == 0 else nc.scalar
                    eng.dma_start(
                        xf[:, 2 * j:2 * j + 2, :],
                        xt.ap()[c, j * 256:(j + 1) * 256, :].rearrange(
                            "(i p) t -> p i t", p=128))
                st[c]["xs"] = xf

            def s01_mm(c):
                coef = ws.tile([W + 1, CHUNK], fp16, tag=f"coef{c}")
                nc.gpsimd.memset(coef[:, :], 1.0)
                cps = [psum_big(), psum_big()]
                xf = st[c]["xs"]
                for ki in range(KH):
                    for n in range(2):
                        nc.tensor.matmul(
                            cps[n][:W, :512], kt_sb[:, ki, :],
                            xf[:, ki, n * 512:(n + 1) * 512],
                            start=(ki == 0), stop=(ki == KH - 1))
                for n in range(2):
                    nc.scalar.copy(coef[:W, n * 512:(n + 1) * 512], cps[n][:W, :512])
                st[c]["coef"] = coef

            # ----- S2: mixer hidden + LN + gelu -> hidT (two-pass LN) -------
            def s2_tiles(c):
                coef = st[c]["coef"]
                hidT = ws.tile([128, KM, CHUNK], fp16, tag=f"hvy{c}")
                st[c]["hidT"] = hidT
                mva = sm.tile([128, NT, 2], f32, tag="mva2", bufs=2, name="mva")
                iva = sm.tile([128, NT], f32, tag="iva2", bufs=2, name="iva")
                hps_l = [None] * NT

                def stats_t(t):
                    hps = psum_big(512)
                    hps_l[t] = hps
                    nc.tensor.matmul(hps[:, :512], coef[:, t * 128:(t + 1) * 128],
                                     w1a_sb[:], start=True, stop=True)
                    st6 = sm.tile([128, 6], f32, tag="st6", bufs=3)
                    nc.vector.bn_stats(st6[:], hps[:, :512])
                    nc.vector.bn_aggr(mva[:, t, :], st6[:])
                    tmp = sm.tile([128, 512], fp16, tag="ntmp", bufs=4)
                    nc.vector.tensor_scalar(tmp[:], hps[:, :512],
                                            mva[:, t, 0:1], None,
                                            op0=Alu.subtract)
                    hps_l[t] = tmp

                def half_iv(hh):
                    sq = sm.tile([128, 4], f32, tag="sq2", name="sq2")
                    nc.scalar.activation(sq[:], mva[:, hh * 4:(hh + 1) * 4, 1],
                                         Act.Sqrt, bias=eps_sb[:])
                    nc.vector.reciprocal_approx_fast(iva[:, hh * 4:(hh + 1) * 4],
                                                     sq[:])

                def norm_t(t):
                    tmp = hps_l[t]
                    nc.gpsimd.tensor_scalar(tmp[:], tmp[:], iva[:, t:t + 1],
                                            None, op0=Alu.mult)
                    nc.sync.dma_start_transpose(hidT[:, :, t * 128:(t + 1) * 128],
                                                tmp[:])

                def gelu_half(hh):
                    for ki in range(KM):
                        sl = hidT[:, ki, hh * 512:(hh + 1) * 512]
                        nc.scalar.activation(sl, sl, Act.Gelu,
                                             scale=gln_sb[:, ki:ki + 1],
                                             bias=bln_sb[:, ki:ki + 1])

                def fin():
                    if debug and c == 0:
                        nc.sync.dma_start(dbg["dhidT"].ap(), hidT[:])
                        nc.sync.dma_start(dbg["dcoef"].ap(), coef[:])

                out = []
                for hh in range(2):
                    for t in range(4 * hh, 4 * hh + 4):
                        out.append(lambda t=t: stats_t(t))
                    out.append(lambda hh=hh: half_iv(hh))
                    for t in range(4 * hh, 4 * hh + 4):
                        out.append(lambda t=t: norm_t(t))
                    out.append(lambda hh=hh: gelu_half(hh))
                return out + [fin]

            # ----- S3: mixed (fp16 matmul) -> mix8 + mixN (staged transpose) --
            def s3_tiles(c):
                hidT = st[c]["hidT"]
                mix8 = ws.tile([128, KH, CHUNK], f8, tag=f"m8{c}")
                mixN = ws.tile([128, NT, H], fp16, tag=f"mN{c}")
                st[c]["mix8"] = mix8
                st[c]["mixN"] = mixN

                def tile_m(m):
                    mps = psum_big()
                    for n in range(2):
                        for ki in range(KM):
                            nc.tensor.matmul(mps[:, n * 512:(n + 1) * 512],
                                             w2_sb[:, ki, m * 128:(m + 1) * 128],
                                             hidT[:, ki, n * 512:(n + 1) * 512],
                                             start=(ki == 0), stop=(ki == KM - 1))
                    mt = sm.tile([128, CHUNK], fp16, tag="mt", bufs=4)
                    nc.scalar.activation(mt[:], mps[:], Act.Identity,
                                         bias=b2_sb[:, m:m + 1])
                    nc.vector.tensor_scalar(mix8[:, m, :], mps[:],
                                            b2_sb[:, m:m + 1], S_M8,
                                            op0=Alu.add, op1=Alu.mult)
                    nc.sync.dma_start_transpose(mixN[:, :, m * 128:(m + 1) * 128],
                                                mt[:])

                def fin():
                    if debug and c == 0:
                        nc.sync.dma_start(dbg["dmix8"].ap(), mix8[:])
                        nc.sync.dma_start(dbg["dmixN"].ap(), mixN[:])

                return [lambda m=m: tile_m(m) for m in range(KH)] + [fin]

            # ----- S4: q/k/v projections (fp8 DoubleRow) -----
            def s4_tiles(c):
                mix8 = st[c]["mix8"]
                qT = ws.tile([128, KH, CHUNK], f8, tag=f"q8{c}")
                kT = ws.tile([128, KH, CHUNK], f8, tag=f"k8{c}")
                vN = ws.tile([128, NT, H], f8, tag=f"hvy{c}")
                st[c]["qT"] = qT
                st[c]["kT"] = kT
                st[c]["vN"] = vN

                def proj_m(dst, wsb, m, on_vec):
                    qps = psum_big()
                    for n in range(2):
                        for g in range(4):
                            nc.tensor.matmul(
                                qps[:, n * 512:(n + 1) * 512],
                                wsb[:, 2 * g:2 * g + 2, m * 128:(m + 1) * 128],
                                mix8[:, 2 * g:2 * g + 2, n * 512:(n + 1) * 512],
                                start=(g == 0), stop=(g == 3), perf_mode=DR)
                    sc = S_Q8 / (S_M8 * S_W8)
                    if on_vec:
                        nc.vector.tensor_scalar(dst[:, m, :], qps[:], sc, None,
                                                op0=Alu.mult)
                    else:
                        nc.scalar.activation(dst[:, m, :], qps[:], Act.Copy,
                                             scale=sc)

                def v_t(t):
                    vps = psum_big()
                    for n in range(2):
                        for g in range(4):
                            nc.tensor.matmul(
                                vps[:, n * 512:(n + 1) * 512],
                                mix8[:, 2 * g:2 * g + 2, t * 128:(t + 1) * 128],
                                wv_sb[:, 2 * g:2 * g + 2, n * 512:(n + 1) * 512],
                                start=(g == 0), stop=(g == 3), perf_mode=DR)
                    nc.vector.tensor_scalar(vN[:, t, :], vps[:],
                                            S_V8 / (S_M8 * S_W8), None,
                                            op0=Alu.mult)

                thunks = []
                for m in range(KH):
                    thunks.append(lambda m=m: proj_m(qT, wq_sb, m, False))
                for m in range(KH):
                    thunks.append(lambda m=m: proj_m(kT, wk_sb, m, c == 1))
                for t in range(NT):
                    thunks.append(lambda t=t: v_t(t))

                def fin():
                    if debug and c == 0:
                        nc.sync.dma_start(dbg["dqT"].ap(), qT[:])
                        nc.sync.dma_start(dbg["dkT"].ap(), kT[:])
                        nc.sync.dma_start(dbg["dvN"].ap(), vN[:])
                thunks.append(fin)
                return thunks

            # ----- S5: attention per head (scores -> exp -> PV+denom -> ocat) --
            def s5_head(c, h, drain):
                qT, kT, vN = st[c]["qT"], st[c]["kT"], st[c]["vN"]
                if h == 0:
                    st[c]["ocat"] = ws.tile([128, NT, HD], fp16,
                                            tag=f"oc{c}", name="ocat")
                    st[c]["et"] = ws.tile([128, KH, CHUNK], f8,
                                          tag=f"xet{c}", name="et")
                    st[c]["otc"] = ws.tile([128, KH, CHUNK], f8,
                                           tag=f"m8{c}", name="otc")
                ocat = st[c]["ocat"]
                otc = st[c]["otc"]
                et = st[c]["et"]
                exp_scale = float(HD ** -0.5) / (S_Q8 * S_Q8)

                for kt in range(NT):
                    stp = psum_st()
                    for qn in range(2):
                        nc.tensor.matmul(
                            stp[:, qn * 512:(qn + 1) * 512],
                            kT[:, 2 * h:2 * h + 2, kt * 128:(kt + 1) * 128],
                            qT[:, 2 * h:2 * h + 2, qn * 512:(qn + 1) * 512],
                            start=True, stop=True, perf_mode=DR)
                    nc.scalar.activation(et[:, kt, :], stp[:], Act.Exp,
                                         scale=exp_scale, bias=lns_sb[:])
                    if kt % 2 == 1:
                        drain(1)
                if debug and c == 0 and h == NUM_HEADS - 1:
                    nc.sync.dma_start(dbg["det"].ap(), et[:])
                # PV with the denominator riding in column HD of the same
                # psum bank (same lhsT -> PE weight-load reuse); this removes
                # the separate denominator pass between exp and PV
                for qt in range(NT):
                    pvs = psum_st()
                    pvp = pvs[:, :HD]
                    for g in range(4):
                        nc.tensor.matmul(
                            pvp[:],
                            et[:, 2 * g:2 * g + 2, qt * 128:(qt + 1) * 128],
                            vN[:, 2 * g:2 * g + 2, h * HD:(h + 1) * HD],
                            start=(g == 0), stop=(g == 3), perf_mode=DR)
                        nc.tensor.matmul(
                            pvs[:, HD:HD + 1],
                            et[:, 2 * g:2 * g + 2, qt * 128:(qt + 1) * 128],
                            ones8[:], start=False, stop=(g == 3),
                            perf_mode=DR, skip_group_check=True)
                    rq = sm.tile([128, 1], f32, tag="rq", bufs=4)
                    nc.vector.reciprocal_approx_fast(rq[:], pvs[:, HD:HD + 1])
                    if debug and c == 0:
                        nc.sync.dma_start(dbg["ddn"].ap()[:, h, qt:qt + 1],
                                          rq[:])
                    dst = ocat[:, qt, :]
                    nc.vector.tensor_scalar(dst, pvp[:], rq[:],
                                            None, op0=Alu.mult)
                    if qt % 2 == 1:
                        drain(1)
                # transpose this head's output into the fp8 feature-major otc
                for qt in range(NT):
                    ot = sm.tile([128, 2, 128], fp16, tag="ott", bufs=4)
                    nc.sync.dma_start_transpose(ot[:], ocat[:, qt, :])
                    nc.gpsimd.tensor_copy(
                        otc[:, 2 * h:2 * h + 2, qt * 128:(qt + 1) * 128], ot[:])
                    if qt % 4 == 3:
                        drain(1)

            def s5_fin(c):
                if debug and c == 0:
                    nc.sync.dma_start(dbg["dotc"].ap(), st[c]["otc"][:])

            # ----- S6+S7: wo proj + residual + LN + output proj -------------
            # Two-pass LN: per-t stats are collected into mvall, then sqrt and
            # reciprocal run once batched (avoids Act Exp<->Sqrt table thrash
            # during the overlapped attention of the other chunk).
            def s67_tiles(c):
                otc8 = st[c]["otc"]  # [128, KH, CHUNK] fp8
                mixN = st[c]["mixN"]
                ych = ws.tile([128, NT, G], f32, tag=f"hvy{c}", name="ych")
                mvall = sm.tile([128, NT, 2], f32, tag="mvall", bufs=2,
                                name="mvall")
                iva = sm.tile([128, NT], f32, tag="iva", bufs=2, name="iva")

                def stats_t(t):
                    ops_ = psum_big()
                    for n in range(2):
                        for g in range(4):
                            nc.tensor.matmul(
                                ops_[:, n * 512:(n + 1) * 512],
                                otc8[:, 2 * g:2 * g + 2, t * 128:(t + 1) * 128],
                                wo_sb[:, 2 * g:2 * g + 2, n * 512:(n + 1) * 512],
                                start=(g == 0), stop=(g == 3), perf_mode=DR)
                    res = mixN[:, t, :]
                    nc.vector.scalar_tensor_tensor(res, ops_[:], INV_WO, res,
                                                   op0=Alu.mult, op1=Alu.add)
                    st6 = sm.tile([128, 2, 6], f32, tag="st6b", bufs=3)
                    for half in range(2):
                        nc.vector.bn_stats(st6[:, half, :],
                                           mixN[:, t, half * 512:(half + 1) * 512])
                    nc.vector.bn_aggr(mvall[:, t, :], st6[:])

                def batch_iv(hh):
                    sq = sm.tile([128, 4], f32, tag="sqb", name="sqb")
                    nc.scalar.activation(sq[:], mvall[:, hh * 4:(hh + 1) * 4, 1],
                                         Act.Sqrt, bias=eps_sb[:])
                    nc.vector.reciprocal_approx_fast(iva[:, hh * 4:(hh + 1) * 4],
                                                     sq[:])

                def norm_t(t):
                    res = mixN[:, t, :]
                    nc.gpsimd.tensor_scalar(res, res, mvall[:, t, 0:1],
                                            iva[:, t:t + 1],
                                            op0=Alu.subtract, op1=Alu.mult)
                    zst = sm.tile([128, KH, 128], fp16, tag="zst", bufs=3)
                    nc.sync.dma_start_transpose(zst[:], res)
                    yps = psum_big()[:, :G]
                    for i in range(KH):
                        nc.tensor.matmul(yps[:], zst[:, i, :],
                                         gw_sb[:, i, :],
                                         start=(i == 0), stop=(i == KH - 1))
                    nc.vector.tensor_tensor(ych[:, t, :], yps[:], bw_sb[:],
                                            op=Alu.add)
                    if debug and c == 0 and t == NT - 1:
                        nc.sync.dma_start(dbg["dres"].ap(), mixN[:])

                def fin():
                    for hh in range(2):
                        nc.sync.dma_start(
                            y.ap()[c, hh * 512:(hh + 1) * 512, :].rearrange(
                                "(t p) g -> p t g", p=128),
                            ych[:, hh * 4:(hh + 1) * 4, :])

                out = []
                for hh in range(2):
                    for t in range(4 * hh, 4 * hh + 4):
                        out.append(lambda t=t: stats_t(t))
                    out.append(lambda hh=hh: batch_iv(hh))
                    for t in range(4 * hh, 4 * hh + 4):
                        out.append(lambda t=t: norm_t(t))
                return out + [fin]

            # ---------------- schedule ----------------
            from collections import deque

            STAGE_MARKS.clear()

            def mark(label):
                n = int(nc.get_next_instruction_name().split("-")[1])
                STAGE_MARKS.append((label, n))

            def mk_drain(q, keep=0):
                state = {"i": 0}

                def drain(n):
                    state["i"] += 1
                    if keep and state["i"] % keep == 0:
                        return
                    for _ in range(min(n, len(q))):
                        q.popleft()()
                return drain

            a, b = 0, 1
            mark("s01a")
            s01_load(a)
            s01_load(b)
            load_big_weights()
            s01_mm(a)
            mark("s2a")
            qa = deque(s2_tiles(a))
            while qa:
                qa.popleft()()
            mark("s01b")
            s01_mm(b)
            # A.S3 + A.S4 with B.S2 interleaved across the whole stream
            mark("s34a+s2b")
            qb = deque(s2_tiles(b))
            for th in s3_tiles(a) + s4_tiles(a):
                th()
                if qb:
                    qb.popleft()()
            while qb:
                qb.popleft()()
            # A.S5 with B.S3 + B.S4 as filler
            mark("s5a+s34b")
            qb = deque(s3_tiles(b) + s4_tiles(b))
            drain_b = mk_drain(qb)
            for h in range(NUM_HEADS):
                s5_head(a, h, drain_b)
            while qb:
                qb.popleft()()
            s5_fin(a)
            # B.S5 with A.S6+S7 as filler
            mark("s5b+s67a")
            qa = deque(s67_tiles(a))
            drain_a = mk_drain(qa)
            for h in range(NUM_HEADS):
                s5_head(b, h, drain_a)
            while qa:
                qa.popleft()()
            s5_fin(b)
            # B tail
            mark("s67b")
            for th in s67_tiles(b):
                th()
            mark("end")

    nc.compile()
    return nc


def _get_compiled():
    global _COMPILED
    if _COMPILED is None:
        _COMPILED = _build()
    return _COMPILED


def _prep_inputs(inputs):
    f32 = np.float32

    def a(name):
        return np.asarray(inputs[name], dtype=f32)

    x = a("x")
    mw = a("mother_wavelets")
    scales = a("scales")
    norm = np.sqrt(np.sum(mw ** 2, axis=2, keepdims=True))
    kern = (mw / np.maximum(norm, 1e-12)) * (1.0 / (1.0 + np.exp(-scales)))
    kern = kern[0, :, :, 0]                      # (W, H)
    kernT = np.ascontiguousarray(kern.T).astype(F16)

    w1a = np.concatenate([a("mix_w1"), a("mix_b1")[None, :]], axis=0).astype(F16)
    gln = np.ascontiguousarray(a("mix_ln_g").reshape(KM, 128).T).astype(f32)
    bln = np.ascontiguousarray(a("mix_ln_b").reshape(KM, 128).T).astype(f32)
    w2 = a("mix_w2").astype(F16)
    b2c = np.ascontiguousarray(a("mix_b2").reshape(KH, 128).T).astype(f32)
    gw = (a("out_ln_g")[:, None] * a("out_w")).astype(F16)
    bw_vec = a("out_ln_b") @ a("out_w") + a("out_b")
    bw = np.tile(bw_vec[None, :], (128, 1)).astype(f32)
    smallw = np.concatenate([gln, bln, b2c, bw], axis=1).astype(f32)

    def to8(w, s):
        ws = w * s
        am = np.abs(ws).max()
        assert am < 224.0, f"fp8 overflow: {am}"
        return ws.astype(F8)

    wq = to8(a("wq"), S_W8)
    wk = to8(a("wk"), S_W8)
    wv = to8(a("wv"), S_W8)
    wo = to8(a("wo"), S_W8)

    shared = {
        "kernt": kernT, "w1a": w1a, "smallw": smallw, "w2": w2,
        "wq8": wq, "wk8": wk, "wv8": wv, "wo8": wo, "gw": gw,
    }

    xc = x.reshape(N_CHUNKS, CHUNK, H)
    xt_all = np.ascontiguousarray(xc.transpose(0, 2, 1)).astype(F16)
    in_maps = []
    for core in range(N_CORES):
        m = dict(shared)
        m["xt"] = np.ascontiguousarray(xt_all[core * CPC:(core + 1) * CPC])
        in_maps.append(m)
    return in_maps


def kernel(**inputs) -> np.ndarray:
    from concourse.bass_utils import run_bass_kernel_spmd

    nc = _get_compiled()
    in_maps = _prep_inputs(inputs)
    res = run_bass_kernel_spmd(nc, in_maps, core_ids=list(range(N_CORES)))
    out = np.concatenate([r["y"] for r in res.results], axis=0)  # (16, CHUNK, G)
    return out.reshape(B, S, G).astype(np.float32)


# revision 79
# speedup vs baseline: 1.0071x; 1.0071x over previous
"""Trainium2 Bass kernel for nn_EntropyLM (wavelet-coeff mixer + chunked MHA + output proj).

Strategy: data-parallel over the 16 independent (batch x chunk) blocks, 2 per
NeuronCore.  The numerically-critical path (wavelet coeffs, mixer, residual
stream, output projection) runs in fp16 on the PE (same speed as bf16, 8x the
mantissa); the error-tolerant bulk (q/k/v projections, attention scores, PV,
attention-out projection) runs in fp8 e4m3 with DoubleRow perf mode, which
contracts K=256 per instruction at 0.5 cycles/row -- 4x bf16 matmul
throughput in the HW cost model.

Per-tensor power-of-two scales keep fp8 operands in [~1, 200]; all scale
corrections are folded into PSUM-evacuation ops that are needed anyway.

Layouts per chunk (CHUNK=1024 tokens, H=1024 features):
  * "T" tensors are feature-major [feat_part, ktile, token]; "N" tensors are
    token-major [token_part, ttile, feat].
  * Attention-out (ocat, token-major fp8) is transposed for the wo matmul by
    viewing fp8 pairs as uint16 through the DMA xbar transpose; the row
    permutation this induces on the contraction index is compensated by
    pre-permuting wo's rows on the host (wo8p).
  * The softmax denominator comes from a 1-column DoubleRow matmul against a
    constant 0.125 vector (reusing the PV lhsT weights); normalization is a
    per-partition scale on the PV evacuation.

The two chunks per core are software-pipelined by emission order: chunk B's
PE-heavy projection tiles are drained as filler between chunk A's Act-bound
attention pieces so the PE never idles waiting on exp().
"""

import numpy as np
import ml_dtypes

B, S, H, G, W = 4, 4096, 1024, 256, 8
CHUNK = 1024
NUM_HEADS = 4
HD = H // NUM_HEADS          # 256 per-head dim
HM = H // 2                  # 512 mixer hidden
N_CHUNKS = B * (S // CHUNK)  # 16 independent chunks
N_CORES = 8
CPC = N_CHUNKS // N_CORES    # 2 chunks per core
NT = CHUNK // 128            # 8 token tiles
KH = H // 128                # 8 feature tiles (H)
KM = HM // 128               # 4 feature tiles (HM)
EPS = 1e-5
BF16 = ml_dtypes.bfloat16
F8 = ml_dtypes.float8_e4m3
F16 = np.float16

# fp8 scales (powers of two; folded into evacuation ops)
S_W8 = 1024.0    # wq/wk/wv/wo weight scale
S_M8 = 64.0      # mix8 activation scale
S_Q8 = 128.0     # q/k fp8 scale
S_V8 = 128.0     # v fp8 scale
S_ET = 16.0      # exp(score) scale
C_ONE = 0.5      # denominator ones value -> ocat = (S_V8/C_ONE) * o = 256*o
S_O8 = S_V8 / C_ONE              # 1024
INV_WO = 1.0 / (S_O8 * S_W8)     # 2^-20

_COMPILED = None
STAGE_MARKS = []


def _build(debug=False):
    import concourse.bass as bass  # noqa: F401
    import concourse.tile as tile
    from concourse import bacc, mybir

    f8 = mybir.dt.float8e4
    u16 = mybir.dt.uint16
    fp16 = mybir.dt.float16
    f32 = mybir.dt.float32
    Alu = mybir.AluOpType
    Act = mybir.ActivationFunctionType
    DR = mybir.MatmulPerfMode.DoubleRow

    nc = bacc.Bacc("TRN2", target_bir_lowering=False, debug=False,
                   enable_asserts=True, num_devices=N_CORES)

    # ---- DRAM tensors (per-core views; same NEFF on all 8 cores) ----
    xt = nc.dram_tensor("xt", [CPC, H, CHUNK], fp16, kind="ExternalInput")
    kernT = nc.dram_tensor("kernt", [H, W], fp16, kind="ExternalInput")
    w1a = nc.dram_tensor("w1a", [W + 1, HM], fp16, kind="ExternalInput")
    smallw = nc.dram_tensor("smallw", [128, 2 * KM + KH + G], f32,
                            kind="ExternalInput")
    w2 = nc.dram_tensor("w2", [HM, H], fp16, kind="ExternalInput")
    wq8 = nc.dram_tensor("wq8", [H, H], f8, kind="ExternalInput")
    wk8 = nc.dram_tensor("wk8", [H, H], f8, kind="ExternalInput")
    wv8 = nc.dram_tensor("wv8", [H, H], f8, kind="ExternalInput")
    wo8 = nc.dram_tensor("wo8", [H, H], f8, kind="ExternalInput")
    gw = nc.dram_tensor("gw", [H, G], fp16, kind="ExternalInput")
    y = nc.dram_tensor("y", [CPC, CHUNK, G], f32, kind="ExternalOutput")
    dbg = {}
    if debug:
        for nm, shp, dt in [
            ("dcoef", [W + 1, CHUNK], fp16),
            ("dhidT", [128, KM, CHUNK], fp16),
            ("dmix8", [128, KH, CHUNK], f8),
            ("dmixN", [128, NT, H], fp16),
            ("dqT", [128, KH, CHUNK], f8),
            ("dkT", [128, KH, CHUNK], f8),
            ("dvN", [128, NT, H], f8),
            ("det", [128, KH, CHUNK], f8),
            ("ddn", [128, NUM_HEADS, NT], f32),
            ("dotc", [128, KH, CHUNK], f8),
            ("dres", [128, NT, H], fp16),
        ]:
            dbg[nm] = nc.dram_tensor(nm, shp, dt, kind="ExternalOutput")

    with tile.TileContext(nc) as tc:
        with (
            tc.tile_pool(name="wp", bufs=1) as wp,
            tc.tile_pool(name="ws", bufs=1) as ws,
            tc.tile_pool(name="sm", bufs=2) as sm,
            tc.tile_pool(name="ps", bufs=1, space="PSUM") as ps,
        ):
            # ---------- persistent weights ----------
            kt_sb = wp.tile([128, KH, W], fp16, tag="ktw")
            nc.sync.dma_start(kt_sb[:], kernT.ap().rearrange("(i p) w -> p i w", p=128))
            w1a_sb = wp.tile([W + 1, HM], fp16, tag="w1a")
            nc.sync.dma_start(w1a_sb[:], w1a.ap())
            smallw_sb = wp.tile([128, 2 * KM + KH + G], f32, tag="smallw")
            gln_sb = smallw_sb[:, 0:KM]
            bln_sb = smallw_sb[:, KM:2 * KM]
            b2_sb = smallw_sb[:, 2 * KM:2 * KM + KH]
            bw_sb = smallw_sb[:, 2 * KM + KH:]
            w2_sb = wp.tile([128, KM, H], fp16, tag="w2s")
            wq_sb = wp.tile([128, KH, H], f8, tag="wq")
            wk_sb = wp.tile([128, KH, H], f8, tag="wk")
            wv_sb = wp.tile([128, KH, H], f8, tag="wv")
            wo_sb = wp.tile([128, KH, H], f8, tag="wo")
            gw_sb = wp.tile([128, KH, G], fp16, tag="gw")

            def load_big_weights():
                # emitted after the x-stream DMAs so they don't delay S1
                nc.scalar.dma_start(smallw_sb[:], smallw.ap())
                nc.sync.dma_start(w2_sb[:],
                                  w2.ap().rearrange("(i p) m -> p i m", p=128))
                nc.scalar.dma_start(wq_sb[:],
                                    wq8.ap().rearrange("(i p) m -> p i m", p=128))
                nc.sync.dma_start(wk_sb[:],
                                  wk8.ap().rearrange("(i p) m -> p i m", p=128))
                nc.scalar.dma_start(wv_sb[:],
                                    wv8.ap().rearrange("(i p) m -> p i m", p=128))
                nc.sync.dma_start(gw_sb[:],
                                  gw.ap().rearrange("(i p) g -> p i g", p=128))
                nc.scalar.dma_start(wo_sb[:],
                                    wo8.ap().rearrange("(i p) m -> p i m", p=128))
            ones8 = wp.tile([128, 2, 1], f8, tag="ones")
            nc.vector.memset(ones8[:], C_ONE)
            eps_sb = wp.tile([128, 1], f32, tag="eps")
            nc.vector.memset(eps_sb[:], EPS)
            lns_sb = wp.tile([128, 1], f32, tag="lns")
            nc.vector.memset(lns_sb[:], float(np.log(S_ET)))

            # ---------- per-chunk state ----------
            st = [dict() for _ in range(CPC)]

            def psum_big(n=1024):
                return ps.tile([128, n], f32, tag="big", bufs=2, name="pbig")

            def psum_st():
                return ps.tile([128, 1024], f32, tag="st", bufs=2, name="pst")

            # ----- S0+S1: stream x (both queues), wavelet coeffs -----
            def s01_load(c):
                xf = ws.tile([128, KH, CHUNK], fp16, tag=f"xet{c}", name="xf")
                for j in range(4):
                    eng = nc.sync if j % 2 == 0 else nc.scalar
                    eng.dma_start(
                        xf[:, 2 * j:2 * j + 2, :],
                        xt.ap()[c, j * 256:(j + 1) * 256, :].rearrange(
                            "(i p) t -> p i t", p=128))
                st[c]["xs"] = xf

            def s01_mm(c):
                coef = ws.tile([W + 1, CHUNK], fp16, tag=f"coef{c}")
                nc.gpsimd.memset(coef[:, :], 1.0)
                cps = [psum_big(), psum_big()]
                xf = st[c]["xs"]
                for ki in range(KH):
                    for n in range(2):
                        nc.tensor.matmul(
                            cps[n][:W, :512], kt_sb[:, ki, :],
                            xf[:, ki, n * 512:(n + 1) * 512],
                            start=(ki == 0), stop=(ki == KH - 1))
                for n in range(2):
                    nc.scalar.copy(coef[:W, n * 512:(n + 1) * 512], cps[n][:W, :512])
                st[c]["coef"] = coef

            # ----- S2: mixer hidden + LN + gelu -> hidT (two-pass LN) -------
            def s2_tiles(c):
                coef = st[c]["coef"]
                hidT = ws.tile([128, KM, CHUNK], fp16, tag=f"hvy{c}")
                st[c]["hidT"] = hidT
                mva = sm.tile([128, NT, 2], f32, tag="mva2", bufs=2, name="mva")
                iva = sm.tile([128, NT], f32, tag="iva2", bufs=2, name="iva")
                hps_l = [None] * NT

                def stats_t(t):
                    hps = psum_big(512)
                    hps_l[t] = hps
                    nc.tensor.matmul(hps[:, :512], coef[:, t * 128:(t + 1) * 128],
                                     w1a_sb[:], start=True, stop=True)
                    st6 = sm.tile([128, 6], f32, tag="st6", bufs=3)
                    nc.vector.bn_stats(st6[:], hps[:, :512])
                    nc.vector.bn_aggr(mva[:, t, :], st6[:])
                    tmp = sm.tile([128, 512], fp16, tag="ntmp", bufs=4)
                    nc.vector.tensor_scalar(tmp[:], hps[:, :512],
                                            mva[:, t, 0:1], None,
                                            op0=Alu.subtract)
                    hps_l[t] = tmp

                def half_iv(hh):
                    sq = sm.tile([128, 4], f32, tag="sq2", name="sq2")
                    nc.scalar.activation(sq[:], mva[:, hh * 4:(hh + 1) * 4, 1],
                                         Act.Sqrt, bias=eps_sb[:])
                    nc.vector.reciprocal_approx_fast(iva[:, hh * 4:(hh + 1) * 4],
                                                     sq[:])

                def norm_t(t):
                    tmp = hps_l[t]
                    nc.gpsimd.tensor_scalar(tmp[:], tmp[:], iva[:, t:t + 1],
                                            None, op0=Alu.mult)
                    nc.sync.dma_start_transpose(hidT[:, :, t * 128:(t + 1) * 128],
                                                tmp[:])

                def gelu_half(hh):
                    for ki in range(KM):
                        sl = hidT[:, ki, hh * 512:(hh + 1) * 512]
                        nc.scalar.activation(sl, sl, Act.Gelu,
                                             scale=gln_sb[:, ki:ki + 1],
                                             bias=bln_sb[:, ki:ki + 1])

                def fin():
                    if debug and c == 0:
                        nc.sync.dma_start(dbg["dhidT"].ap(), hidT[:])
                        nc.sync.dma_start(dbg["dcoef"].ap(), coef[:])

                out = []
                for hh in range(2):
                    for t in range(4 * hh, 4 * hh + 4):
                        out.append(lambda t=t: stats_t(t))
                    out.append(lambda hh=hh: half_iv(hh))
                    for t in range(4 * hh, 4 * hh + 4):
                        out.append(lambda t=t: norm_t(t))
                    out.append(lambda hh=hh: gelu_half(hh))
                return out + [fin]

            # ----- S3: mixed (fp16 matmul) -> mix8 + mixN (staged transpose) --
            def s3_tiles(c):
                hidT = st[c]["hidT"]
                mix8 = ws.tile([128, KH, CHUNK], f8, tag=f"m8{c}")
                mixN = ws.tile([128, NT, H], fp16, tag=f"mN{c}")
                st[c]["mix8"] = mix8
                st[c]["mixN"] = mixN

                def tile_m(m):
                    mps = psum_big()
                    for n in range(2):
                        for ki in range(KM):
                            nc.tensor.matmul(mps[:, n * 512:(n + 1) * 512],
                                             w2_sb[:, ki, m * 128:(m + 1) * 128],
                                             hidT[:, ki, n * 512:(n + 1) * 512],
                                             start=(ki == 0), stop=(ki == KM - 1))
                    mt = sm.tile([128, CHUNK], fp16, tag="mt", bufs=4)
                    nc.scalar.activation(mt[:], mps[:], Act.Identity,
                                         bias=b2_sb[:, m:m + 1])
                    nc.vector.tensor_scalar(mix8[:, m, :], mps[:],
                                            b2_sb[:, m:m + 1], S_M8,
                                            op0=Alu.add, op1=Alu.mult)
                    nc.sync.dma_start_transpose(mixN[:, :, m * 128:(m + 1) * 128],
                                                mt[:])

                def fin():
                    if debug and c == 0:
                        nc.sync.dma_start(dbg["dmix8"].ap(), mix8[:])
                        nc.sync.dma_start(dbg["dmixN"].ap(), mixN[:])

                return [lambda m=m: tile_m(m) for m in range(KH)] + [fin]

            # ----- S4: q/k/v projections (fp8 DoubleRow) -----
            def s4_tiles(c):
                mix8 = st[c]["mix8"]
                qT = ws.tile([128, KH, CHUNK], f8, tag=f"q8{c}")
                kT = ws.tile([128, KH, CHUNK], f8, tag=f"k8{c}")
                vN = ws.tile([128, NT, H], f8, tag=f"hvy{c}")
                st[c]["qT"] = qT
                st[c]["kT"] = kT
                st[c]["vN"] = vN

                def proj_m(dst, wsb, m, on_vec):
                    qps = psum_big()
                    for n in range(2):
                        for g in range(4):
                            nc.tensor.matmul(
                                qps[:, n * 512:(n + 1) * 512],
                                wsb[:, 2 * g:2 * g + 2, m * 128:(m + 1) * 128],
                                mix8[:, 2 * g:2 * g + 2, n * 512:(n + 1) * 512],
                                start=(g == 0), stop=(g == 3), perf_mode=DR)
                    sc = S_Q8 / (S_M8 * S_W8)
                    if on_vec:
                        nc.vector.tensor_scalar(dst[:, m, :], qps[:], sc, None,
                                                op0=Alu.mult)
                    else:
                        nc.scalar.activation(dst[:, m, :], qps[:], Act.Copy,
                                             scale=sc)

                def v_t(t):
                    vps = psum_big()
                    for n in range(2):
                        for g in range(4):
                            nc.tensor.matmul(
                                vps[:, n * 512:(n + 1) * 512],
                                mix8[:, 2 * g:2 * g + 2, t * 128:(t + 1) * 128],
                                wv_sb[:, 2 * g:2 * g + 2, n * 512:(n + 1) * 512],
                                start=(g == 0), stop=(g == 3), perf_mode=DR)
                    nc.vector.tensor_scalar(vN[:, t, :], vps[:],
                                            S_V8 / (S_M8 * S_W8), None,
                                            op0=Alu.mult)

                thunks = []
                for m in range(KH):
                    thunks.append(lambda m=m: proj_m(qT, wq_sb, m, False))
                for m in range(KH):
                    thunks.append(lambda m=m: proj_m(kT, wk_sb, m, c == 1))
                for t in range(NT):
                    thunks.append(lambda t=t: v_t(t))

                def fin():
                    if debug and c == 0:
                        nc.sync.dma_start(dbg["dqT"].ap(), qT[:])
                        nc.sync.dma_start(dbg["dkT"].ap(), kT[:])
                        nc.sync.dma_start(dbg["dvN"].ap(), vN[:])
                thunks.append(fin)
                return thunks

            # ----- S5: attention per head (scores -> exp -> PV+denom -> ocat) --
            def s5_head(c, h, drain):
                qT, kT, vN = st[c]["qT"], st[c]["kT"], st[c]["vN"]
                if h == 0:
                    st[c]["ocat"] = ws.tile([128, NT, HD], fp16,
                                            tag=f"oc{c}", name="ocat")
                    st[c]["et"] = ws.tile([128, KH, CHUNK], f8,
                                          tag=f"xet{c}", name="et")
                    st[c]["otc"] = ws.tile([128, KH, CHUNK], f8,
                                           tag=f"m8{c}", name="otc")
                ocat = st[c]["ocat"]
                otc = st[c]["otc"]
                et = st[c]["et"]
                exp_scale = float(HD ** -0.5) / (S_Q8 * S_Q8)

                for kt in range(NT):
                    stp = psum_st()
                    for qn in range(2):
                        nc.tensor.matmul(
                            stp[:, qn * 512:(qn + 1) * 512],
                            kT[:, 2 * h:2 * h + 2, kt * 128:(kt + 1) * 128],
                            qT[:, 2 * h:2 * h + 2, qn * 512:(qn + 1) * 512],
                            start=True, stop=True, perf_mode=DR)
                    nc.scalar.activation(et[:, kt, :], stp[:], Act.Exp,
                                         scale=exp_scale, bias=lns_sb[:])
                # drains kept out of the kt loop: the exp stream runs at pure
                # scores-ring pace with no filler matmuls between scores
                drain(4)
                if debug and c == 0 and h == NUM_HEADS - 1:
                    nc.sync.dma_start(dbg["det"].ap(), et[:])
                # PV with the denominator riding in column HD of the same
                # psum bank (same lhsT -> PE weight-load reuse); this removes
                # the separate denominator pass between exp and PV
                for qt in range(NT):
                    pvs = psum_st()
                    pvp = pvs[:, :HD]
                    for g in range(4):
                        nc.tensor.matmul(
                            pvp[:],
                            et[:, 2 * g:2 * g + 2, qt * 128:(qt + 1) * 128],
                            vN[:, 2 * g:2 * g + 2, h * HD:(h + 1) * HD],
                            start=(g == 0), stop=(g == 3), perf_mode=DR)
                        nc.tensor.matmul(
                            pvs[:, HD:HD + 1],
                            et[:, 2 * g:2 * g + 2, qt * 128:(qt + 1) * 128],
                            ones8[:], start=False, stop=(g == 3),
                            perf_mode=DR, skip_group_check=True)
                    rq = sm.tile([128, 1], f32, tag="rq", bufs=4)
                    nc.vector.reciprocal_approx_fast(rq[:], pvs[:, HD:HD + 1])
                    if debug and c == 0:
                        nc.sync.dma_start(dbg["ddn"].ap()[:, h, qt:qt + 1],
                                          rq[:])
                    dst = ocat[:, qt, :]
                    nc.vector.tensor_scalar(dst, pvp[:], rq[:],
                                            None, op0=Alu.mult)
                    if qt % 2 == 1:
                        drain(1)
                # transpose this head's output into the fp8 feature-major otc
                for qt in range(NT):
                    ot = sm.tile([128, 2, 128], fp16, tag="ott", bufs=4)
                    nc.sync.dma_start_transpose(ot[:], ocat[:, qt, :])
                    nc.gpsimd.tensor_copy(
                        otc[:, 2 * h:2 * h + 2, qt * 128:(qt + 1) * 128], ot[:])
                drain(2)

            def s5_fin(c):
                if debug and c == 0:
                    nc.sync.dma_start(dbg["dotc"].ap(), st[c]["otc"][:])

            # ----- S6+S7: wo proj + residual + LN + output proj -------------
            # Two-pass LN: per-t stats are collected into mvall, then sqrt and
            # reciprocal run once batched (avoids Act Exp<->Sqrt table thrash
            # during the overlapped attention of the other chunk).
            def s67_tiles(c):
                otc8 = st[c]["otc"]  # [128, KH, CHUNK] fp8
                mixN = st[c]["mixN"]
                ych = ws.tile([128, NT, G], f32, tag=f"hvy{c}", name="ych")
                mvall = sm.tile([128, NT, 2], f32, tag="mvall", bufs=2,
                                name="mvall")
                iva = sm.tile([128, NT], f32, tag="iva", bufs=2, name="iva")

                def stats_t(t):
                    ops_ = psum_big()
                    for n in range(2):
                        for g in range(4):
                            nc.tensor.matmul(
                                ops_[:, n * 512:(n + 1) * 512],
                                otc8[:, 2 * g:2 * g + 2, t * 128:(t + 1) * 128],
                                wo_sb[:, 2 * g:2 * g + 2, n * 512:(n + 1) * 512],
                                start=(g == 0), stop=(g == 3), perf_mode=DR)
                    res = mixN[:, t, :]
                    nc.vector.scalar_tensor_tensor(res, ops_[:], INV_WO, res,
                                                   op0=Alu.mult, op1=Alu.add)
                    st6 = sm.tile([128, 2, 6], f32, tag="st6b", bufs=3)
                    for half in range(2):
                        nc.vector.bn_stats(st6[:, half, :],
                                           mixN[:, t, half * 512:(half + 1) * 512])
                    nc.vector.bn_aggr(mvall[:, t, :], st6[:])

                def batch_iv(hh):
                    sq = sm.tile([128, 4], f32, tag="sqb", name="sqb")
                    nc.scalar.activation(sq[:], mvall[:, hh * 4:(hh + 1) * 4, 1],
                                         Act.Sqrt, bias=eps_sb[:])
                    nc.vector.reciprocal_approx_fast(iva[:, hh * 4:(hh + 1) * 4],
                                                     sq[:])

                def norm_t(t):
                    res = mixN[:, t, :]
                    nc.gpsimd.tensor_scalar(res, res, mvall[:, t, 0:1],
                                            iva[:, t:t + 1],
                                            op0=Alu.subtract, op1=Alu.mult)
                    zst = sm.tile([128, KH, 128], fp16, tag="zst", bufs=3)
                    nc.sync.dma_start_transpose(zst[:], res)
                    yps = psum_big()[:, :G]
                    for i in range(KH):
                        nc.tensor.matmul(yps[:], zst[:, i, :],
                                         gw_sb[:, i, :],
                                         start=(i == 0), stop=(i == KH - 1))
                    nc.vector.tensor_tensor(ych[:, t, :], yps[:], bw_sb[:],
                                            op=Alu.add)
                    if debug and c == 0 and t == NT - 1:
                        nc.sync.dma_start(dbg["dres"].ap(), mixN[:])

                def fin():
                    for hh in range(2):
                        nc.sync.dma_start(
                            y.ap()[c, hh * 512:(hh + 1) * 512, :].rearrange(
                                "(t p) g -> p t g", p=128),
                            ych[:, hh * 4:(hh + 1) * 4, :])

                out = []
                for hh in range(2):
                    for t in range(4 * hh, 4 * hh + 4):
                        out.append(lambda t=t: stats_t(t))
                    out.append(lambda hh=hh: batch_iv(hh))
                    for t in range(4 * hh, 4 * hh + 4):
                        out.append(lambda t=t: norm_t(t))
                return out + [fin]

            # ---------------- schedule ----------------
            from collections import deque

            STAGE_MARKS.clear()

            def mark(label):
                n = int(nc.get_next_instruction_name().split("-")[1])
                STAGE_MARKS.append((label, n))

            def mk_drain(q, keep=0):
                state = {"i": 0}

                def drain(n):
                    state["i"] += 1
                    if keep and state["i"] % keep == 0:
                        return
                    for _ in range(min(n, len(q))):
                        q.popleft()()
                return drain

            a, b = 0, 1
            mark("s01a")
            s01_load(a)
            s01_load(b)
            load_big_weights()
            s01_mm(a)
            mark("s2a")
            qa = deque(s2_tiles(a))
            while qa:
                qa.popleft()()
            mark("s01b")
            s01_mm(b)
            # A.S3 + A.S4 with B.S2 interleaved across the whole stream
            mark("s34a+s2b")
            qb = deque(s2_tiles(b))
            for th in s3_tiles(a) + s4_tiles(a):
                th()
                if qb:
                    qb.popleft()()
            while qb:
                qb.popleft()()
            # A.S5 with B.S3 + B.S4 as filler
            mark("s5a+s34b")
            qb = deque(s3_tiles(b) + s4_tiles(b))
            drain_b = mk_drain(qb)
            for h in range(NUM_HEADS):
                s5_head(a, h, drain_b)
            while qb:
                qb.popleft()()
            s5_fin(a)
            # B.S5 with A.S6+S7 as filler
            mark("s5b+s67a")
            qa = deque(s67_tiles(a))
            drain_a = mk_drain(qa)
            for h in range(NUM_HEADS):
                s5_head(b, h, drain_a)
            while qa:
                qa.popleft()()
            s5_fin(b)
            # B tail
            mark("s67b")
            for th in s67_tiles(b):
                th()
            mark("end")

    nc.compile()
    return nc


def _get_compiled():
    global _COMPILED
    if _COMPILED is None:
        _COMPILED = _build()
    return _COMPILED


def _prep_inputs(inputs):
    f32 = np.float32

    def a(name):
        return np.asarray(inputs[name], dtype=f32)

    x = a("x")
    mw = a("mother_wavelets")
    scales = a("scales")
    norm = np.sqrt(np.sum(mw ** 2, axis=2, keepdims=True))
    kern = (mw / np.maximum(norm, 1e-12)) * (1.0 / (1.0 + np.exp(-scales)))
    kern = kern[0, :, :, 0]                      # (W, H)
    kernT = np.ascontiguousarray(kern.T).astype(F16)

    w1a = np.concatenate([a("mix_w1"), a("mix_b1")[None, :]], axis=0).astype(F16)
    gln = np.ascontiguousarray(a("mix_ln_g").reshape(KM, 128).T).astype(f32)
    bln = np.ascontiguousarray(a("mix_ln_b").reshape(KM, 128).T).astype(f32)
    w2 = a("mix_w2").astype(F16)
    b2c = np.ascontiguousarray(a("mix_b2").reshape(KH, 128).T).astype(f32)
    gw = (a("out_ln_g")[:, None] * a("out_w")).astype(F16)
    bw_vec = a("out_ln_b") @ a("out_w") + a("out_b")
    bw = np.tile(bw_vec[None, :], (128, 1)).astype(f32)
    smallw = np.concatenate([gln, bln, b2c, bw], axis=1).astype(f32)

    def to8(w, s):
        ws = w * s
        am = np.abs(ws).max()
        assert am < 224.0, f"fp8 overflow: {am}"
        return ws.astype(F8)

    wq = to8(a("wq"), S_W8)
    wk = to8(a("wk"), S_W8)
    wv = to8(a("wv"), S_W8)
    wo = to8(a("wo"), S_W8)

    shared = {
        "kernt": kernT, "w1a": w1a, "smallw": smallw, "w2": w2,
        "wq8": wq, "wk8": wk, "wv8": wv, "wo8": wo, "gw": gw,
    }

    xc = x.reshape(N_CHUNKS, CHUNK, H)
    xt_all = np.ascontiguousarray(xc.transpose(0, 2, 1)).astype(F16)
    in_maps = []
    for core in range(N_CORES):
        m = dict(shared)
        m["xt"] = np.ascontiguousarray(xt_all[core * CPC:(core + 1) * CPC])
        in_maps.append(m)
    return in_maps


def kernel(**inputs) -> np.ndarray:
    from concourse.bass_utils import run_bass_kernel_spmd

    nc = _get_compiled()
    in_maps = _prep_inputs(inputs)
    res = run_bass_kernel_spmd(nc, in_maps, core_ids=list(range(N_CORES)))
    out = np.concatenate([r["y"] for r in res.results], axis=0)  # (16, CHUNK, G)
    return out.reshape(B, S, G).astype(np.float32)


# revision 80
# speedup vs baseline: 1.0227x; 1.0154x over previous
"""Trainium2 Bass kernel for nn_EntropyLM (wavelet-coeff mixer + chunked MHA + output proj).

Strategy: data-parallel over the 16 independent (batch x chunk) blocks, 2 per
NeuronCore.  The numerically-critical path (wavelet coeffs, mixer, residual
stream, output projection) runs in fp16 on the PE (same speed as bf16, 8x the
mantissa); the error-tolerant bulk (q/k/v projections, attention scores, PV,
attention-out projection) runs in fp8 e4m3 with DoubleRow perf mode, which
contracts K=256 per instruction at 0.5 cycles/row -- 4x bf16 matmul
throughput in the HW cost model.

Per-tensor power-of-two scales keep fp8 operands in [~1, 200]; all scale
corrections are folded into PSUM-evacuation ops that are needed anyway.

Layouts per chunk (CHUNK=1024 tokens, H=1024 features):
  * "T" tensors are feature-major [feat_part, ktile, token]; "N" tensors are
    token-major [token_part, ttile, feat].
  * Attention-out (ocat, token-major fp8) is transposed for the wo matmul by
    viewing fp8 pairs as uint16 through the DMA xbar transpose; the row
    permutation this induces on the contraction index is compensated by
    pre-permuting wo's rows on the host (wo8p).
  * The softmax denominator comes from a 1-column DoubleRow matmul against a
    constant 0.125 vector (reusing the PV lhsT weights); normalization is a
    per-partition scale on the PV evacuation.

The two chunks per core are software-pipelined by emission order: chunk B's
PE-heavy projection tiles are drained as filler between chunk A's Act-bound
attention pieces so the PE never idles waiting on exp().
"""

import numpy as np
import ml_dtypes

B, S, H, G, W = 4, 4096, 1024, 256, 8
CHUNK = 1024
NUM_HEADS = 4
HD = H // NUM_HEADS          # 256 per-head dim
HM = H // 2                  # 512 mixer hidden
N_CHUNKS = B * (S // CHUNK)  # 16 independent chunks
N_CORES = 8
CPC = N_CHUNKS // N_CORES    # 2 chunks per core
NT = CHUNK // 128            # 8 token tiles
KH = H // 128                # 8 feature tiles (H)
KM = HM // 128               # 4 feature tiles (HM)
EPS = 1e-5
BF16 = ml_dtypes.bfloat16
F8 = ml_dtypes.float8_e4m3
F16 = np.float16

# fp8 scales (powers of two; folded into evacuation ops)
S_W8 = 1024.0    # wq/wk/wv/wo weight scale
S_M8 = 64.0      # mix8 activation scale
S_Q8 = 128.0     # q/k fp8 scale
S_V8 = 128.0     # v fp8 scale
S_ET = 16.0      # exp(score) scale
C_ONE = 0.5      # denominator ones value -> ocat = (S_V8/C_ONE) * o = 256*o
S_O8 = S_V8 / C_ONE              # 1024
INV_WO = 1.0 / (S_O8 * S_W8)     # 2^-20

_COMPILED = None
STAGE_MARKS = []


def _build(debug=False):
    import concourse.bass as bass  # noqa: F401
    import concourse.tile as tile
    from concourse import bacc, mybir

    f8 = mybir.dt.float8e4
    u16 = mybir.dt.uint16
    fp16 = mybir.dt.float16
    f32 = mybir.dt.float32
    Alu = mybir.AluOpType
    Act = mybir.ActivationFunctionType
    DR = mybir.MatmulPerfMode.DoubleRow

    nc = bacc.Bacc("TRN2", target_bir_lowering=False, debug=False,
                   enable_asserts=True, num_devices=N_CORES)

    # ---- DRAM tensors (per-core views; same NEFF on all 8 cores) ----
    xt = nc.dram_tensor("xt", [CPC, H, CHUNK], fp16, kind="ExternalInput")
    kernT = nc.dram_tensor("kernt", [H, W], fp16, kind="ExternalInput")
    w1a = nc.dram_tensor("w1a", [W + 1, HM], fp16, kind="ExternalInput")
    smallw = nc.dram_tensor("smallw", [128, 2 * KM + KH + G], f32,
                            kind="ExternalInput")
    w2 = nc.dram_tensor("w2", [HM, H], fp16, kind="ExternalInput")
    wq8 = nc.dram_tensor("wq8", [H, H], f8, kind="ExternalInput")
    wk8 = nc.dram_tensor("wk8", [H, H], f8, kind="ExternalInput")
    wv8 = nc.dram_tensor("wv8", [H, H], f8, kind="ExternalInput")
    wo8 = nc.dram_tensor("wo8", [H, H], f8, kind="ExternalInput")
    gw = nc.dram_tensor("gw", [H, G], fp16, kind="ExternalInput")
    y = nc.dram_tensor("y", [CPC, CHUNK, G], f32, kind="ExternalOutput")
    dbg = {}
    if debug:
        for nm, shp, dt in [
            ("dcoef", [W + 1, CHUNK], fp16),
            ("dhidT", [128, KM, CHUNK], fp16),
            ("dmix8", [128, KH, CHUNK], f8),
            ("dmixN", [128, NT, H], fp16),
            ("dqT", [128, KH, CHUNK], f8),
            ("dkT", [128, KH, CHUNK], f8),
            ("dvN", [128, NT, H], f8),
            ("det", [128, KH, CHUNK], f8),
            ("ddn", [128, NUM_HEADS, NT], f32),
            ("dotc", [128, KH, CHUNK], f8),
            ("dres", [128, NT, H], fp16),
        ]:
            dbg[nm] = nc.dram_tensor(nm, shp, dt, kind="ExternalOutput")

    with tile.TileContext(nc) as tc:
        with (
            tc.tile_pool(name="wp", bufs=1) as wp,
            tc.tile_pool(name="ws", bufs=1) as ws,
            tc.tile_pool(name="sm", bufs=2) as sm,
            tc.tile_pool(name="ps", bufs=1, space="PSUM") as ps,
        ):
            # ---------- persistent weights ----------
            kt_sb = wp.tile([128, KH, W], fp16, tag="ktw")
            nc.sync.dma_start(kt_sb[:], kernT.ap().rearrange("(i p) w -> p i w", p=128))
            w1a_sb = wp.tile([W + 1, HM], fp16, tag="w1a")
            nc.sync.dma_start(w1a_sb[:], w1a.ap())
            smallw_sb = wp.tile([128, 2 * KM + KH + G], f32, tag="smallw")
            gln_sb = smallw_sb[:, 0:KM]
            bln_sb = smallw_sb[:, KM:2 * KM]
            b2_sb = smallw_sb[:, 2 * KM:2 * KM + KH]
            bw_sb = smallw_sb[:, 2 * KM + KH:]
            w2_sb = wp.tile([128, KM, H], fp16, tag="w2s")
            wq_sb = wp.tile([128, KH, H], f8, tag="wq")
            wk_sb = wp.tile([128, KH, H], f8, tag="wk")
            wv_sb = wp.tile([128, KH, H], f8, tag="wv")
            wo_sb = wp.tile([128, KH, H], f8, tag="wo")
            gw_sb = wp.tile([128, KH, G], fp16, tag="gw")

            def load_big_weights():
                # emitted after the x-stream DMAs so they don't delay S1
                nc.scalar.dma_start(smallw_sb[:], smallw.ap())
                nc.sync.dma_start(w2_sb[:],
                                  w2.ap().rearrange("(i p) m -> p i m", p=128))
                nc.scalar.dma_start(wq_sb[:],
                                    wq8.ap().rearrange("(i p) m -> p i m", p=128))
                nc.sync.dma_start(wk_sb[:],
                                  wk8.ap().rearrange("(i p) m -> p i m", p=128))
                nc.scalar.dma_start(wv_sb[:],
                                    wv8.ap().rearrange("(i p) m -> p i m", p=128))
                nc.sync.dma_start(gw_sb[:],
                                  gw.ap().rearrange("(i p) g -> p i g", p=128))
                nc.scalar.dma_start(wo_sb[:],
                                    wo8.ap().rearrange("(i p) m -> p i m", p=128))
            ones8 = wp.tile([128, 2, 1], f8, tag="ones")
            nc.vector.memset(ones8[:], C_ONE)
            eps_sb = wp.tile([128, 1], f32, tag="eps")
            nc.vector.memset(eps_sb[:], EPS)
            lns_sb = wp.tile([128, 1], f32, tag="lns")
            nc.vector.memset(lns_sb[:], float(np.log(S_ET)))

            # ---------- per-chunk state ----------
            st = [dict() for _ in range(CPC)]

            def psum_big(n=1024):
                return ps.tile([128, n], f32, tag="big", bufs=2, name="pbig")

            def psum_st():
                return ps.tile([128, 1024], f32, tag="st", bufs=2, name="pst")

            # ----- S0+S1: stream x (both queues), wavelet coeffs -----
            def s01_load(c):
                xf = ws.tile([128, KH, CHUNK], fp16, tag=f"xet{c}", name="xf")
                for j in range(4):
                    eng = nc.sync if j % 2 == 0 else nc.scalar
                    eng.dma_start(
                        xf[:, 2 * j:2 * j + 2, :],
                        xt.ap()[c, j * 256:(j + 1) * 256, :].rearrange(
                            "(i p) t -> p i t", p=128))
                st[c]["xs"] = xf

            def s01_mm(c):
                coef = ws.tile([W + 1, CHUNK], fp16, tag=f"coef{c}")
                nc.gpsimd.memset(coef[:, :], 1.0)
                cps = [psum_big(), psum_big()]
                xf = st[c]["xs"]
                for ki in range(KH):
                    for n in range(2):
                        nc.tensor.matmul(
                            cps[n][:W, :512], kt_sb[:, ki, :],
                            xf[:, ki, n * 512:(n + 1) * 512],
                            start=(ki == 0), stop=(ki == KH - 1))
                for n in range(2):
                    nc.scalar.copy(coef[:W, n * 512:(n + 1) * 512], cps[n][:W, :512])
                st[c]["coef"] = coef

            # ----- S2: mixer hidden + LN + gelu -> hidT (two-pass LN) -------
            def s2_tiles(c):
                coef = st[c]["coef"]
                hidT = ws.tile([128, KM, CHUNK], fp16, tag=f"hvy{c}")
                st[c]["hidT"] = hidT
                mva = sm.tile([128, NT, 2], f32, tag="mva2", bufs=2, name="mva")
                iva = sm.tile([128, NT], f32, tag="iva2", bufs=2, name="iva")
                hps_l = [None] * NT

                def stats_t(t):
                    hps = psum_big(512)
                    hps_l[t] = hps
                    nc.tensor.matmul(hps[:, :512], coef[:, t * 128:(t + 1) * 128],
                                     w1a_sb[:], start=True, stop=True)
                    st6 = sm.tile([128, 6], f32, tag="st6", bufs=3)
                    nc.vector.bn_stats(st6[:], hps[:, :512])
                    nc.vector.bn_aggr(mva[:, t, :], st6[:])
                    tmp = sm.tile([128, 512], fp16, tag="ntmp", bufs=4)
                    nc.vector.tensor_scalar(tmp[:], hps[:, :512],
                                            mva[:, t, 0:1], None,
                                            op0=Alu.subtract)
                    hps_l[t] = tmp

                def half_iv(hh):
                    sq = sm.tile([128, 4], f32, tag="sq2", name="sq2")
                    nc.scalar.activation(sq[:], mva[:, hh * 4:(hh + 1) * 4, 1],
                                         Act.Sqrt, bias=eps_sb[:])
                    nc.vector.reciprocal_approx_fast(iva[:, hh * 4:(hh + 1) * 4],
                                                     sq[:])

                def norm_t(t):
                    tmp = hps_l[t]
                    nc.gpsimd.tensor_scalar(tmp[:], tmp[:], iva[:, t:t + 1],
                                            None, op0=Alu.mult)
                    nc.sync.dma_start_transpose(hidT[:, :, t * 128:(t + 1) * 128],
                                                tmp[:])

                def gelu_half(hh):
                    for ki in range(KM):
                        sl = hidT[:, ki, hh * 512:(hh + 1) * 512]
                        nc.scalar.activation(sl, sl, Act.Gelu,
                                             scale=gln_sb[:, ki:ki + 1],
                                             bias=bln_sb[:, ki:ki + 1])

                def fin():
                    if debug and c == 0:
                        nc.sync.dma_start(dbg["dhidT"].ap(), hidT[:])
                        nc.sync.dma_start(dbg["dcoef"].ap(), coef[:])

                out = []
                for hh in range(2):
                    for t in range(4 * hh, 4 * hh + 4):
                        out.append(lambda t=t: stats_t(t))
                    out.append(lambda hh=hh: half_iv(hh))
                    for t in range(4 * hh, 4 * hh + 4):
                        out.append(lambda t=t: norm_t(t))
                    out.append(lambda hh=hh: gelu_half(hh))
                return out + [fin]

            # ----- S3: mixed (fp16 matmul) -> mix8 + mixN (staged transpose) --
            def s3_tiles(c):
                hidT = st[c]["hidT"]
                mix8 = ws.tile([128, KH, CHUNK], f8, tag=f"m8{c}")
                mixN = ws.tile([128, NT, H], fp16, tag=f"mN{c}")
                st[c]["mix8"] = mix8
                st[c]["mixN"] = mixN

                def tile_m(m):
                    mps = psum_big()
                    for n in range(2):
                        for ki in range(KM):
                            nc.tensor.matmul(mps[:, n * 512:(n + 1) * 512],
                                             w2_sb[:, ki, m * 128:(m + 1) * 128],
                                             hidT[:, ki, n * 512:(n + 1) * 512],
                                             start=(ki == 0), stop=(ki == KM - 1))
                    mt = sm.tile([128, CHUNK], fp16, tag="mt", bufs=4)
                    nc.scalar.activation(mt[:], mps[:], Act.Identity,
                                         bias=b2_sb[:, m:m + 1])
                    nc.vector.tensor_scalar(mix8[:, m, :], mps[:],
                                            b2_sb[:, m:m + 1], S_M8,
                                            op0=Alu.add, op1=Alu.mult)
                    nc.sync.dma_start_transpose(mixN[:, :, m * 128:(m + 1) * 128],
                                                mt[:])

                def fin():
                    if debug and c == 0:
                        nc.sync.dma_start(dbg["dmix8"].ap(), mix8[:])
                        nc.sync.dma_start(dbg["dmixN"].ap(), mixN[:])

                return [lambda m=m: tile_m(m) for m in range(KH)] + [fin]

            # ----- S4: q/k/v projections (fp8 DoubleRow) -----
            def s4_tiles(c):
                mix8 = st[c]["mix8"]
                qT = ws.tile([128, KH, CHUNK], f8, tag=f"q8{c}")
                kT = ws.tile([128, KH, CHUNK], f8, tag=f"k8{c}")
                vN = ws.tile([128, NT, H], f8, tag=f"hvy{c}")
                st[c]["qT"] = qT
                st[c]["kT"] = kT
                st[c]["vN"] = vN

                def proj_m(dst, wsb, m, on_vec):
                    qps = psum_big()
                    for n in range(2):
                        for g in range(4):
                            nc.tensor.matmul(
                                qps[:, n * 512:(n + 1) * 512],
                                wsb[:, 2 * g:2 * g + 2, m * 128:(m + 1) * 128],
                                mix8[:, 2 * g:2 * g + 2, n * 512:(n + 1) * 512],
                                start=(g == 0), stop=(g == 3), perf_mode=DR)
                    sc = S_Q8 / (S_M8 * S_W8)
                    if on_vec:
                        nc.vector.tensor_scalar(dst[:, m, :], qps[:], sc, None,
                                                op0=Alu.mult)
                    else:
                        nc.scalar.activation(dst[:, m, :], qps[:], Act.Copy,
                                             scale=sc)

                def v_t(t):
                    vps = psum_big()
                    for n in range(2):
                        for g in range(4):
                            nc.tensor.matmul(
                                vps[:, n * 512:(n + 1) * 512],
                                mix8[:, 2 * g:2 * g + 2, t * 128:(t + 1) * 128],
                                wv_sb[:, 2 * g:2 * g + 2, n * 512:(n + 1) * 512],
                                start=(g == 0), stop=(g == 3), perf_mode=DR)
                    nc.vector.tensor_scalar(vN[:, t, :], vps[:],
                                            S_V8 / (S_M8 * S_W8), None,
                                            op0=Alu.mult)

                thunks = []
                for m in range(KH):
                    thunks.append(lambda m=m: proj_m(qT, wq_sb, m, False))
                for m in range(KH):
                    thunks.append(lambda m=m: proj_m(kT, wk_sb, m, c == 1))
                for t in range(NT):
                    thunks.append(lambda t=t: v_t(t))

                def fin():
                    if debug and c == 0:
                        nc.sync.dma_start(dbg["dqT"].ap(), qT[:])
                        nc.sync.dma_start(dbg["dkT"].ap(), kT[:])
                        nc.sync.dma_start(dbg["dvN"].ap(), vN[:])
                thunks.append(fin)
                return thunks

            # ----- S5: attention per head (scores -> exp -> PV+denom -> ocat) --
            def s5_head(c, h, drain):
                qT, kT, vN = st[c]["qT"], st[c]["kT"], st[c]["vN"]
                if h == 0:
                    st[c]["ocat"] = ws.tile([128, NT, HD], fp16,
                                            tag=f"oc{c}", name="ocat")
                    st[c]["et"] = ws.tile([128, KH, CHUNK], f8,
                                          tag=f"xet{c}", name="et")
                    st[c]["otc"] = ws.tile([128, KH, CHUNK], f8,
                                           tag=f"m8{c}", name="otc")
                ocat = st[c]["ocat"]
                otc = st[c]["otc"]
                et = st[c]["et"]
                exp_scale = float(HD ** -0.5) / (S_Q8 * S_Q8)

                for kt in range(NT):
                    stp = psum_st()
                    for qn in range(2):
                        nc.tensor.matmul(
                            stp[:, qn * 512:(qn + 1) * 512],
                            kT[:, 2 * h:2 * h + 2, kt * 128:(kt + 1) * 128],
                            qT[:, 2 * h:2 * h + 2, qn * 512:(qn + 1) * 512],
                            start=True, stop=True, perf_mode=DR)
                    nc.scalar.activation(et[:, kt, :], stp[:], Act.Exp,
                                         scale=exp_scale, bias=lns_sb[:])
                # drains kept out of the kt loop: the exp stream runs at pure
                # scores-ring pace with no filler matmuls between scores
                if debug and c == 0 and h == NUM_HEADS - 1:
                    nc.sync.dma_start(dbg["det"].ap(), et[:])
                # PV with the denominator riding in column HD of the same
                # psum bank (same lhsT -> PE weight-load reuse); this removes
                # the separate denominator pass between exp and PV
                for qt in range(NT):
                    pvs = psum_st()
                    pvp = pvs[:, :HD]
                    for g in range(4):
                        nc.tensor.matmul(
                            pvp[:],
                            et[:, 2 * g:2 * g + 2, qt * 128:(qt + 1) * 128],
                            vN[:, 2 * g:2 * g + 2, h * HD:(h + 1) * HD],
                            start=(g == 0), stop=(g == 3), perf_mode=DR)
                        nc.tensor.matmul(
                            pvs[:, HD:HD + 1],
                            et[:, 2 * g:2 * g + 2, qt * 128:(qt + 1) * 128],
                            ones8[:], start=False, stop=(g == 3),
                            perf_mode=DR, skip_group_check=True)
                    rq = sm.tile([128, 1], f32, tag="rq", bufs=4)
                    nc.vector.reciprocal_approx_fast(rq[:], pvs[:, HD:HD + 1])
                    if debug and c == 0:
                        nc.sync.dma_start(dbg["ddn"].ap()[:, h, qt:qt + 1],
                                          rq[:])
                    dst = ocat[:, qt, :]
                    nc.vector.tensor_scalar(dst, pvp[:], rq[:],
                                            None, op0=Alu.mult)
                    drain(1)
                # transpose this head's output into the fp8 feature-major otc
                for qt in range(NT):
                    ot = sm.tile([128, 2, 128], fp16, tag="ott", bufs=4)
                    nc.sync.dma_start_transpose(ot[:], ocat[:, qt, :])
                    nc.gpsimd.tensor_copy(
                        otc[:, 2 * h:2 * h + 2, qt * 128:(qt + 1) * 128], ot[:])
                drain(2)

            def s5_fin(c):
                if debug and c == 0:
                    nc.sync.dma_start(dbg["dotc"].ap(), st[c]["otc"][:])

            # ----- S6+S7: wo proj + residual + LN + output proj -------------
            # Two-pass LN: per-t stats are collected into mvall, then sqrt and
            # reciprocal run once batched (avoids Act Exp<->Sqrt table thrash
            # during the overlapped attention of the other chunk).
            def s67_tiles(c):
                otc8 = st[c]["otc"]  # [128, KH, CHUNK] fp8
                mixN = st[c]["mixN"]
                ych = ws.tile([128, NT, G], f32, tag=f"hvy{c}", name="ych")
                mvall = sm.tile([128, NT, 2], f32, tag="mvall", bufs=2,
                                name="mvall")
                iva = sm.tile([128, NT], f32, tag="iva", bufs=2, name="iva")

                def stats_t(t):
                    ops_ = psum_big()
                    for n in range(2):
                        for g in range(4):
                            nc.tensor.matmul(
                                ops_[:, n * 512:(n + 1) * 512],
                                otc8[:, 2 * g:2 * g + 2, t * 128:(t + 1) * 128],
                                wo_sb[:, 2 * g:2 * g + 2, n * 512:(n + 1) * 512],
                                start=(g == 0), stop=(g == 3), perf_mode=DR)
                    res = mixN[:, t, :]
                    nc.vector.scalar_tensor_tensor(res, ops_[:], INV_WO, res,
                                                   op0=Alu.mult, op1=Alu.add)
                    st6 = sm.tile([128, 2, 6], f32, tag="st6b", bufs=3)
                    for half in range(2):
                        nc.vector.bn_stats(st6[:, half, :],
                                           mixN[:, t, half * 512:(half + 1) * 512])
                    nc.vector.bn_aggr(mvall[:, t, :], st6[:])

                def batch_iv(hh):
                    sq = sm.tile([128, 4], f32, tag="sqb", name="sqb")
                    nc.scalar.activation(sq[:], mvall[:, hh * 4:(hh + 1) * 4, 1],
                                         Act.Sqrt, bias=eps_sb[:])
                    nc.vector.reciprocal_approx_fast(iva[:, hh * 4:(hh + 1) * 4],
                                                     sq[:])

                def norm_t(t):
                    res = mixN[:, t, :]
                    nc.gpsimd.tensor_scalar(res, res, mvall[:, t, 0:1],
                                            iva[:, t:t + 1],
                                            op0=Alu.subtract, op1=Alu.mult)
                    zst = sm.tile([128, KH, 128], fp16, tag="zst", bufs=3)
                    nc.sync.dma_start_transpose(zst[:], res)
                    yps = psum_big()[:, :G]
                    for i in range(KH):
                        nc.tensor.matmul(yps[:], zst[:, i, :],
                                         gw_sb[:, i, :],
                                         start=(i == 0), stop=(i == KH - 1))
                    nc.vector.tensor_tensor(ych[:, t, :], yps[:], bw_sb[:],
                                            op=Alu.add)
                    if debug and c == 0 and t == NT - 1:
                        nc.sync.dma_start(dbg["dres"].ap(), mixN[:])

                def fin():
                    for hh in range(2):
                        nc.sync.dma_start(
                            y.ap()[c, hh * 512:(hh + 1) * 512, :].rearrange(
                                "(t p) g -> p t g", p=128),
                            ych[:, hh * 4:(hh + 1) * 4, :])

                out = []
                for hh in range(2):
                    for t in range(4 * hh, 4 * hh + 4):
                        out.append(lambda t=t: stats_t(t))
                    out.append(lambda hh=hh: batch_iv(hh))
                    for t in range(4 * hh, 4 * hh + 4):
                        out.append(lambda t=t: norm_t(t))
                return out + [fin]

            # ---------------- schedule ----------------
            from collections import deque

            STAGE_MARKS.clear()

            def mark(label):
                n = int(nc.get_next_instruction_name().split("-")[1])
                STAGE_MARKS.append((label, n))

            def mk_drain(q, keep=0):
                state = {"i": 0}

                def drain(n):
                    state["i"] += 1
                    if keep and state["i"] % keep == 0:
                        return
                    for _ in range(min(n, len(q))):
                        q.popleft()()
                return drain

            a, b = 0, 1
            mark("s01a")
            s01_load(a)
            s01_load(b)
            load_big_weights()
            s01_mm(a)
            mark("s2a")
            qa = deque(s2_tiles(a))
            while qa:
                qa.popleft()()
            mark("s01b")
            s01_mm(b)
            # A.S3 + A.S4 with B.S2 interleaved across the whole stream
            mark("s34a+s2b")
            qb = deque(s2_tiles(b))
            for th in s3_tiles(a) + s4_tiles(a):
                th()
                if qb:
                    qb.popleft()()
            while qb:
                qb.popleft()()
            # A.S5 with B.S3 + B.S4 as filler
            mark("s5a+s34b")
            qb = deque(s3_tiles(b) + s4_tiles(b))
            drain_b = mk_drain(qb)
            for h in range(NUM_HEADS):
                s5_head(a, h, drain_b)
            while qb:
                qb.popleft()()
            s5_fin(a)
            # B.S5 with A.S6+S7 as filler
            mark("s5b+s67a")
            qa = deque(s67_tiles(a))
            drain_a = mk_drain(qa)
            for h in range(NUM_HEADS):
                s5_head(b, h, drain_a)
            while qa:
                qa.popleft()()
            s5_fin(b)
            # B tail
            mark("s67b")
            for th in s67_tiles(b):
                th()
            mark("end")

    nc.compile()
    return nc


def _get_compiled():
    global _COMPILED
    if _COMPILED is None:
        _COMPILED = _build()
    return _COMPILED


def _prep_inputs(inputs):
    f32 = np.float32

    def a(name):
        return np.asarray(inputs[name], dtype=f32)

    x = a("x")
    mw = a("mother_wavelets")
    scales = a("scales")
    norm = np.sqrt(np.sum(mw ** 2, axis=2, keepdims=True))
    kern = (mw / np.maximum(norm, 1e-12)) * (1.0 / (1.0 + np.exp(-scales)))
    kern = kern[0, :, :, 0]                      # (W, H)
    kernT = np.ascontiguousarray(kern.T).astype(F16)

    w1a = np.concatenate([a("mix_w1"), a("mix_b1")[None, :]], axis=0).astype(F16)
    gln = np.ascontiguousarray(a("mix_ln_g").reshape(KM, 128).T).astype(f32)
    bln = np.ascontiguousarray(a("mix_ln_b").reshape(KM, 128).T).astype(f32)
    w2 = a("mix_w2").astype(F16)
    b2c = np.ascontiguousarray(a("mix_b2").reshape(KH, 128).T).astype(f32)
    gw = (a("out_ln_g")[:, None] * a("out_w")).astype(F16)
    bw_vec = a("out_ln_b") @ a("out_w") + a("out_b")
    bw = np.tile(bw_vec[None, :], (128, 1)).astype(f32)
    smallw = np.concatenate([gln, bln, b2c, bw], axis=1).astype(f32)

    def to8(w, s):
        ws = w * s
        am = np.abs(ws).max()
        assert am < 224.0, f"fp8 overflow: {am}"
        return ws.astype(F8)

    wq = to8(a("wq"), S_W8)
    wk = to8(a("wk"), S_W8)
    wv = to8(a("wv"), S_W8)
    wo = to8(a("wo"), S_W8)

    shared = {
        "kernt": kernT, "w1a": w1a, "smallw": smallw, "w2": w2,
        "wq8": wq, "wk8": wk, "wv8": wv, "wo8": wo, "gw": gw,
    }

    xc = x.reshape(N_CHUNKS, CHUNK, H)
    xt_all = np.ascontiguousarray(xc.transpose(0, 2, 1)).astype(F16)
    in_maps = []
    for core in range(N_CORES):
        m = dict(shared)
        m["xt"] = np.ascontiguousarray(xt_all[core * CPC:(core + 1) * CPC])
        in_maps.append(m)
    return in_maps


def kernel(**inputs) -> np.ndarray:
    from concourse.bass_utils import run_bass_kernel_spmd

    nc = _get_compiled()
    in_maps = _prep_inputs(inputs)
    res = run_bass_kernel_spmd(nc, in_maps, core_ids=list(range(N_CORES)))
    out = np.concatenate([r["y"] for r in res.results], axis=0)  # (16, CHUNK, G)
    return out.reshape(B, S, G).astype(np.float32)
